# revision 7
# baseline (speedup 1.0000x reference)
"""Bahdanau attention on 8 Trainium2 NeuronCores (Bass/Tile, SPMD).

Math (per batch b):
    c[b]      = W1 @ hidden[b] + attn_b                  (prologue, tiny)
    energyT   = tanh(W2 @ enc[b].T + c[b][:, None])      [H, S] (PE + ACT)
    scores    = v @ energyT                              [S]    (PE, M=1)
    attn_w    = softmax(scores)                          (DVE/ACT, [1, S] rows)
    context   = attn_w @ enc[b]                          [H]    (PE, M=1)

Sharding: data-parallel over batch (32 -> 4 per core), weights replicated.
All matmul operands are float32r (TF32-like; full PE rate at N>=256).
Batch-outer loop: softmax + context pass of batch b overlap the energy
GEMM of batch b+1 (context pass emission is software-pipelined by one batch).
"""

import numpy as np

import concourse.bass as bass
import concourse.mybir as mybir
import concourse.tile as tile
from concourse import bacc
from concourse.bass_utils import run_bass_kernel_spmd
from concourse.masks import make_identity

B, S, H = 32, 2048, 1024
NCORES = 8
BL = B // NCORES      # local batches per core
SC = 512              # seq chunk for pass 1 (matmul moving dim)
F32 = mybir.dt.float32
F32R = mybir.dt.float32r
AX = mybir.AxisListType
AF = mybir.ActivationFunctionType


def build(bl=BL, s=S, h=H, sc=SC):
    """Build the per-core Bass module. Dims: h%128==0, s%sc==0, s%128==0,
    sc<=512."""
    jm = h // 128          # number of 128-blocks of the hidden dim
    nsc = s // sc          # pass-1 seq chunks
    nst = s // 128         # pass-2 seq tiles
    nhc = (h + 511) // 512  # 512-chunks of h (hid_proj / context N-chunks)

    def hchunks():
        return [(i * 512, min(512, h - i * 512)) for i in range(nhc)]

    nc = bacc.Bacc("TRN2", target_bir_lowering=False, debug=False)

    encT = nc.declare_dram_parameter("encT", [bl, nsc, jm, 128, sc], F32R, isOutput=False)
    encN = nc.declare_dram_parameter("encN", [bl, nst, 128, h], F32R, isOutput=False)
    w2t = nc.declare_dram_parameter("w2t", [jm, 128, h], F32R, isOutput=False)
    w1t = nc.declare_dram_parameter("w1t", [jm, 128, h], F32R, isOutput=False)
    hidT = nc.declare_dram_parameter("hidT", [jm, 128, bl], F32R, isOutput=False)
    vT = nc.declare_dram_parameter("vT", [jm, 128], F32R, isOutput=False)
    abT = nc.declare_dram_parameter("abT", [jm, 128], F32, isOutput=False)
    ctx_out = nc.declare_dram_parameter("ctx_out", [bl, h], F32, isOutput=True)
    aw_out = nc.declare_dram_parameter("aw_out", [bl, s], F32, isOutput=True)

    with tile.TileContext(nc) as tc:
        with (
            tc.tile_pool(name="persist", bufs=1) as sb1,
            tc.tile_pool(name="et", bufs=3) as pool_et,
            tc.tile_pool(name="th", bufs=2) as pool_th,
            tc.tile_pool(name="en", bufs=6) as pool_en,
            tc.tile_pool(name="rows", bufs=2) as pool_rw,
            tc.tile_pool(name="ps_e", bufs=2, space="PSUM") as pool_e,
            tc.tile_pool(name="ps_s", bufs=2, space="PSUM") as pool_s,
            tc.tile_pool(name="ps_m", bufs=2, space="PSUM") as pool_m,
            tc.tile_pool(name="ps_c", bufs=1, space="PSUM") as pool_c,
        ):
            # ---- constants / weights resident in SBUF ----
            ident = sb1.tile([128, 128], F32)
            make_identity(nc, ident)

            w2t_sb = sb1.tile([128, jm, h], F32R)
            nc.sync.dma_start(out=w2t_sb, in_=w2t.ap().rearrange("j p h -> p j h"))
            hidT_sb = sb1.tile([128, jm, bl], F32R)
            nc.sync.dma_start(out=hidT_sb, in_=hidT.ap().rearrange("j p b -> p j b"))
            vT_sb = sb1.tile([128, jm], F32R)
            nc.sync.dma_start(out=vT_sb, in_=vT.ap().rearrange("j p -> p j"))
            ab_sb = sb1.tile([128, jm], F32)
            nc.sync.dma_start(out=ab_sb, in_=abT.ap().rearrange("j p -> p j"))

            c_t = sb1.tile([128, jm, bl], F32)

            # ---- prologue: c = W1 @ hidden^T + attn_b, laid out [h(part), b] ----
            with tc.tile_pool(name="w1", bufs=2) as w1pool:
                hp_sb = pool_rw.tile([bl, h], F32, tag="hp")
                pchunks = [
                    (i * 256, min(256, h - i * 256)) for i in range((h + 255) // 256)
                ]
                for off, ln in pchunks:
                    w1c = w1pool.tile([128, jm, ln], F32R, tag="w1c", name="w1c")
                    nc.sync.dma_start(
                        out=w1c,
                        in_=w1t.ap()[:, :, off : off + ln].rearrange("j p h -> p j h"),
                    )
                    hp_ps = pool_m.tile([bl, ln], F32, tag="misc")
                    for j in range(jm):
                        nc.tensor.matmul(
                            hp_ps,
                            hidT_sb[:, j, :],
                            w1c[:, j, :],
                            start=(j == 0),
                            stop=(j == jm - 1),
                        )
                    nc.vector.tensor_copy(hp_sb[:, off : off + ln], hp_ps)
                for m in range(jm):
                    tr_ps = pool_m.tile([128, bl], F32, tag="misc")
                    nc.tensor.transpose(
                        tr_ps, hp_sb[:, m * 128 : (m + 1) * 128], ident[:bl, :bl]
                    )
                    nc.vector.tensor_scalar(
                        out=c_t[:, m, :],
                        in0=tr_ps,
                        scalar1=ab_sb[:, m : m + 1],
                        scalar2=None,
                        op0=mybir.AluOpType.add,
                    )

            def pass1(b):
                """Energy GEMM + tanh + scores for batch b; returns wT tile
                [128, nst] (f32r) of softmaxed attention weights, transposed."""
                srow = pool_rw.tile([1, s], F32, tag="srow", name="srow")
                for sci in range(nsc):
                    et = pool_et.tile([128, jm, sc], F32R, name="et")
                    nc.sync.dma_start(
                        out=et, in_=encT.ap()[b, sci].rearrange("j p s -> p j s")
                    )
                    th = pool_th.tile([128, jm, sc], F32R, name="th")
                    s_ps = pool_s.tile([1, sc], F32, tag="scores", name="s_ps")
                    for m in range(jm):
                        e_ps = pool_e.tile([128, sc], F32, tag="energy", name="e_ps")
                        for j in range(jm):
                            nc.tensor.matmul(
                                e_ps,
                                w2t_sb[:, j, m * 128 : (m + 1) * 128],
                                et[:, j, :],
                                start=(j == 0),
                                stop=(j == jm - 1),
                            )
                        nc.scalar.activation(
                            th[:, m, :], e_ps, AF.Tanh, bias=c_t[:, m, b : b + 1]
                        )
                        # scores matmul for the previous m-block: gives ACT a
                        # full energy-group of slack before PE consumes tanh m
                        if m > 0:
                            nc.tensor.matmul(
                                s_ps,
                                vT_sb[:, m - 1 : m],
                                th[:, m - 1, :],
                                start=(m - 1 == 0),
                                stop=False,
                            )
                    nc.tensor.matmul(
                        s_ps, vT_sb[:, jm - 1 : jm], th[:, jm - 1, :],
                        start=(jm == 1), stop=True,
                    )
                    nc.vector.tensor_copy(srow[:, sci * sc : (sci + 1) * sc], s_ps)

                # softmax over the [1, s] row
                mx = pool_rw.tile([1, 1], F32, tag="mx", name="mx")
                nc.vector.reduce_max(mx, srow, axis=AX.X)
                mxn = pool_rw.tile([1, 1], F32, tag="mxn", name="mxn")
                nc.vector.tensor_scalar_mul(mxn, mx, -1.0)
                aw_row = pool_rw.tile([1, s], F32, tag="aw", name="aw_row")
                sm = pool_rw.tile([1, 1], F32, tag="sm", name="sm")
                nc.scalar.activation(aw_row, srow, AF.Exp, bias=mxn, accum_out=sm)
                rec = pool_rw.tile([1, 1], F32, tag="rec", name="rec")
                nc.vector.reciprocal(rec, sm)
                nc.vector.tensor_scalar_mul(aw_row, aw_row, rec)
                nc.sync.dma_start(out=aw_out.ap()[b : b + 1, :], in_=aw_row)

                wTb = pool_rw.tile([128, nst], F32R, tag="wT", name="wTb")
                for st in range(nst):
                    tr_ps = pool_m.tile([128, 1], F32, tag="misc", name="tr_ps")
                    nc.tensor.transpose(
                        tr_ps, aw_row[:, st * 128 : (st + 1) * 128], ident[:1, :1]
                    )
                    nc.vector.tensor_copy(wTb[:, st : st + 1], tr_ps)
                return wTb

            def pass2(b, wTb):
                """context[b] = attn_w[b] @ enc[b]."""
                ctx_ps = []
                for ci, (off, ln) in enumerate(hchunks()):
                    ctx_tile = pool_c.tile([1, ln], F32, tag=f"ctx{ci}", name=f"ctx{ci}")
                    ctx_ps.append(ctx_tile)
                for st in range(nst):
                    en = pool_en.tile([128, h], F32R, name="en")
                    nc.sync.dma_start(out=en, in_=encN.ap()[b, st])
                    for ci, (off, ln) in enumerate(hchunks()):
                        nc.tensor.matmul(
                            ctx_ps[ci],
                            wTb[:, st : st + 1],
                            en[:, off : off + ln],
                            start=(st == 0),
                            stop=(st == nst - 1),
                        )
                crow = pool_rw.tile([1, h], F32, tag="ctxrow", name="crow")
                for ci, (off, ln) in enumerate(hchunks()):
                    nc.vector.tensor_copy(crow[:, off : off + ln], ctx_ps[ci])
                nc.sync.dma_start(out=ctx_out.ap()[b : b + 1, :], in_=crow)

            # software pipeline: pass2(b) emitted after pass1(b+1)
            pending = None
            for b in range(bl):
                wTb = pass1(b)
                if pending is not None:
                    pass2(*pending)
                pending = (b, wTb)
            pass2(*pending)

    nc.finalize()
    return nc


def shard_inputs(hidden, encoder_outputs, attn_W, attn_b, v_W, bl=BL, sc=SC):
    """Host-side prep: slice per core + lay out for the device program."""
    hidden = np.ascontiguousarray(np.asarray(hidden, dtype=np.float32))
    enc = np.asarray(encoder_outputs, dtype=np.float32)
    attn_W = np.asarray(attn_W, dtype=np.float32)
    attn_b = np.ascontiguousarray(np.asarray(attn_b, dtype=np.float32))
    v_W = np.ascontiguousarray(np.asarray(v_W, dtype=np.float32))
    s, b_total, h = enc.shape
    jm = h // 128
    ncores = b_total // bl

    wt = np.ascontiguousarray(attn_W.T)          # [2h, h]
    w1t = np.ascontiguousarray(wt[:h].reshape(jm, 128, h))
    w2t = np.ascontiguousarray(wt[h:].reshape(jm, 128, h))
    vT = v_W[0].reshape(jm, 128)
    abT = attn_b.reshape(jm, 128)

    in_maps = []
    for c in range(ncores):
        bs = slice(c * bl, (c + 1) * bl)
        enc_c = enc[:, bs, :]                    # [s, bl, h]
        encN = np.ascontiguousarray(enc_c.transpose(1, 0, 2))  # [bl, s, h]
        encT = np.ascontiguousarray(
            encN.reshape(bl, s // sc, sc, jm, 128).transpose(0, 1, 3, 4, 2)
        )                                        # [bl, nsc, jm, 128, sc]
        hidT = np.ascontiguousarray(hidden[bs].T.reshape(jm, 128, bl))
        in_maps.append(
            {
                "encT": encT,
                "encN": encN.reshape(bl, s // 128, 128, h),
                "w2t": w2t,
                "w1t": w1t,
                "hidT": hidT,
                "vT": vT,
                "abT": abT,
            }
        )
    return in_maps


_NC_CACHE = {}


def _get_nc():
    if "nc" not in _NC_CACHE:
        _NC_CACHE["nc"] = build()
    return _NC_CACHE["nc"]


def kernel(hidden, encoder_outputs, attn_W, attn_b, v_W):
    nc = _get_nc()
    in_maps = shard_inputs(hidden, encoder_outputs, attn_W, attn_b, v_W)
    res = run_bass_kernel_spmd(nc, in_maps, list(range(NCORES)))
    context = np.concatenate([r["ctx_out"] for r in res.results], axis=0)
    attn_weights = np.concatenate([r["aw_out"] for r in res.results], axis=0)
    return context, attn_weights


# revision 22
# speedup vs baseline: 145.4347x; 145.4347x over previous
"""Bahdanau attention on 8 Trainium2 NeuronCores (Bass/Tile, SPMD).

Single-pass design (per batch b, per seq chunk):
    energyT = W2 @ enc_chunk.T             [128h x 512s] x 8 m-blocks  (PE, f32r)
    tanhT   = tanh(energyT + c[b])         c = W1 @ hid + attn_b, host-computed;
                                           per-partition ACT bias, PSUM->SBUF
    scores  = v @ tanhT                    M=1 PE matmul, K-accumulated
    w       = exp(scores - M)              M = sum|v| >= max|scores| (fixed
                                           softmax shift -- no running max)
    wb      = ones @ w                     PE broadcast to 128 partitions
    acc    += enc_chunk.T * wb (sum s)     DVE tensor_tensor_reduce, chained
                                           via the initial-value operand
  end of batch: l = sum exp(srow - M); context = acc/l; attn_w = exp(srow-M)/l

One streaming pass over enc (encT layout only), no second enc read, no
collectives. Data-parallel over batch (32 -> 4 per core), weights replicated.
All matmul operands are float32r (TF32-like; full PE rate at N>=256).
PE-blocking ops that depend on slow ACT chains (wb, recip broadcast) are
deferred one energy-group to keep the PE queue stall-free.
"""

from collections import deque

import numpy as np

import concourse.bass as bass
import concourse.mybir as mybir
import concourse.tile as tile
from concourse import bacc
from concourse.bass_utils import run_bass_kernel_spmd

B, S, H = 32, 2048, 1024
NCORES = 8
BL = B // NCORES      # local batches per core
SC = 512              # seq chunk for pass 1 (matmul moving dim)
F32 = mybir.dt.float32
F32R = mybir.dt.float32r
AX = mybir.AxisListType
AF = mybir.ActivationFunctionType


def build(bl=BL, s=S, h=H, sc=SC):
    """Build the per-core Bass module. Dims: h%128==0, s%sc==0, sc<=512."""
    jm = h // 128          # number of 128-blocks of the hidden dim
    nsc = s // sc          # seq chunks

    nc = bacc.Bacc("TRN2", target_bir_lowering=False, debug=False)

    encT = nc.declare_dram_parameter("encT", [bl, nsc, jm, 128, sc], F32R, isOutput=False)
    w2t = nc.declare_dram_parameter("w2t", [jm, 128, h], F32R, isOutput=False)
    vT = nc.declare_dram_parameter("vT", [jm, 128], F32R, isOutput=False)
    cT = nc.declare_dram_parameter("cT", [jm, 128, bl], F32, isOutput=False)
    negM = nc.declare_dram_parameter("negM", [1, 1], F32, isOutput=False)
    ctx_out = nc.declare_dram_parameter("ctx_out", [bl, 128, jm], F32, isOutput=True)
    aw_out = nc.declare_dram_parameter("aw_out", [bl, s], F32, isOutput=True)
    l_out = nc.declare_dram_parameter("l_out", [bl, 1], F32, isOutput=True)

    with tile.TileContext(nc) as tc:
        with (
            tc.tile_pool(name="persist", bufs=1) as sb1,
            tc.tile_pool(name="et", bufs=4) as pool_et,
            tc.tile_pool(name="th", bufs=2) as pool_th,
            tc.tile_pool(name="rows", bufs=2) as pool_rw,
            tc.tile_pool(name="ps_e", bufs=2, space="PSUM") as pool_e,
            tc.tile_pool(name="ps_s", bufs=2, space="PSUM") as pool_s,
            tc.tile_pool(name="ps_w", bufs=2, space="PSUM") as pool_w,
        ):
            # ---- weights / constants resident in SBUF ----
            w2t_sb = sb1.tile([128, jm, h], F32R)
            nc.sync.dma_start(out=w2t_sb, in_=w2t.ap().rearrange("j p h -> p j h"))
            vT_sb = sb1.tile([128, jm], F32R)
            nc.scalar.dma_start(out=vT_sb, in_=vT.ap().rearrange("j p -> p j"))
            c_t = sb1.tile([128, jm, bl], F32)
            nc.scalar.dma_start(out=c_t, in_=cT.ap().rearrange("j p b -> p j b"))
            negm_sb = sb1.tile([1, 1], F32)
            nc.scalar.dma_start(out=negm_sb, in_=negM.ap())
            ones_f32 = sb1.tile([1, 128], F32)
            nc.vector.memset(ones_f32, 1.0)
            ones_sb = sb1.tile([1, 128], F32R)
            nc.vector.tensor_copy(ones_sb, ones_f32)

            # PE ops gated on slow ACT/DVE chains are emitted one
            # energy-group later so the in-order PE queue never stalls.
            deferred = deque()

            def drain_one():
                if deferred:
                    deferred.popleft()()

            def flush_deferred():
                while deferred:
                    deferred.popleft()()

            def energy_chunk(b, sci, srow):
                """Emit energy GEMM + tanh + scores for (b, chunk); returns
                (et, wexp) used by the deferred accumulation step."""
                et = pool_et.tile([128, jm, sc], F32R, name="et")
                nc.sync.dma_start(
                    out=et, in_=encT.ap()[b, sci].rearrange("j p s -> p j s")
                )
                th = pool_th.tile([128, jm, sc], F32R, name="th")
                s_ps = pool_s.tile([1, sc], F32, tag="scores", name="s_ps")
                for m in range(jm):
                    e_ps = pool_e.tile([128, sc], F32, tag="energy", name="e_ps")
                    for j in range(jm):
                        nc.tensor.matmul(
                            e_ps,
                            w2t_sb[:, j, m * 128 : (m + 1) * 128],
                            et[:, j, :],
                            start=(j == 0),
                            stop=(j == jm - 1),
                        )
                    if m > 0:
                        nc.tensor.matmul(
                            s_ps, vT_sb[:, m - 1 : m], th[:, m - 1, :],
                            start=(m - 1 == 0), stop=False,
                        )
                        drain_one()
                    nc.scalar.activation(
                        th[:, m, :], e_ps, AF.Tanh, bias=c_t[:, m, b : b + 1]
                    )
                nc.tensor.matmul(
                    s_ps, vT_sb[:, jm - 1 : jm], th[:, jm - 1, :],
                    start=(jm == 1), stop=True,
                )
                nc.vector.tensor_copy(srow[:, sci * sc : (sci + 1) * sc], s_ps)
                # unnormalized softmax weights for this chunk
                wexp = pool_rw.tile([1, sc], F32R, tag="wexp", name="wexp")
                nc.scalar.activation(wexp, s_ps, AF.Exp, bias=negm_sb)
                return et, wexp

            def accum_chunk(et, wexp, acc, scratch):
                """Deferred: broadcast w to 128 partitions (PE), then fold the
                chunk into the context accumulator (DVE mult/reduce/add)."""
                wb_ps = pool_w.tile([128, sc], F32, tag="wb", name="wb_ps")

                def emit():
                    nc.tensor.matmul(wb_ps, ones_sb, wexp, start=True, stop=True)
                    for j in range(jm):
                        nc.vector.tensor_mul(
                            scratch, et[:, j, :].bitcast(F32), wb_ps
                        )
                        part = pool_rw.tile([128, 1], F32, tag="part", name="part")
                        nc.vector.reduce_sum(part, scratch, axis=AX.X)
                        nc.vector.tensor_add(
                            acc[:, j : j + 1], acc[:, j : j + 1], part
                        )

                deferred.append(emit)

            def finish_batch(b, srow, acc):
                """Softmax epilogue. The context stays unnormalized on device;
                the host divides by l (exact fp32, one scalar per batch)."""
                aw_row = pool_rw.tile([1, s], F32, tag="aw", name="aw_row")
                l_sum = pool_rw.tile([1, 1], F32, tag="lsum", name="l_sum")
                nc.scalar.activation(aw_row, srow, AF.Exp, bias=negm_sb, accum_out=l_sum)
                rec = pool_rw.tile([1, 1], F32, tag="rec", name="rec")
                nc.vector.reciprocal(rec, l_sum)
                nc.vector.tensor_scalar_mul(aw_row, aw_row, rec)
                nc.sync.dma_start(out=aw_out.ap()[b : b + 1, :], in_=aw_row)
                nc.sync.dma_start(out=l_out.ap()[b : b + 1, :], in_=l_sum)

                def emit():
                    nc.sync.dma_start(out=ctx_out.ap()[b], in_=acc)

                deferred.append(emit)

            for b in range(bl):
                srow = pool_rw.tile([1, s], F32, tag="srow", name="srow")
                scratch = pool_rw.tile([128, sc], F32, tag="scr", name="scratch")
                acc = pool_rw.tile([128, jm], F32, tag="acc", name="acc")
                nc.vector.memset(acc, 0.0)
                for sci in range(nsc):
                    et, wexp = energy_chunk(b, sci, srow)
                    accum_chunk(et, wexp, acc, scratch)
                finish_batch(b, srow, acc)
            flush_deferred()

    nc.finalize()
    return nc


def shard_inputs(hidden, encoder_outputs, attn_W, attn_b, v_W, bl=BL, sc=SC):
    """Host-side prep: slice per core + lay out for the device program.
    Also computes the tiny decoder projection c = W1 @ hid + attn_b (0.05%
    of total FLOPs) and the softmax shift M = sum|v|."""
    hidden = np.ascontiguousarray(np.asarray(hidden, dtype=np.float32))
    enc = np.asarray(encoder_outputs, dtype=np.float32)
    attn_W = np.asarray(attn_W, dtype=np.float32)
    attn_b = np.ascontiguousarray(np.asarray(attn_b, dtype=np.float32))
    v_W = np.ascontiguousarray(np.asarray(v_W, dtype=np.float32))
    s, b_total, h = enc.shape
    jm = h // 128
    ncores = b_total // bl

    w2t = np.ascontiguousarray(attn_W.T[h:].reshape(jm, 128, h))
    vT = v_W[0].reshape(jm, 128)
    c_all = hidden @ attn_W[:, :h].T + attn_b          # [b_total, h]
    neg_m = -np.abs(v_W).sum(dtype=np.float32).reshape(1, 1)

    in_maps = []
    for c in range(ncores):
        bs = slice(c * bl, (c + 1) * bl)
        enc_c = enc[:, bs, :]                          # [s, bl, h]
        encT = np.ascontiguousarray(
            enc_c.reshape(s // sc, sc, bl, jm, 128).transpose(2, 0, 3, 4, 1)
        )                                              # [bl, nsc, jm, 128, sc]
        cT = np.ascontiguousarray(c_all[bs].T.reshape(jm, 128, bl))
        in_maps.append(
            {"encT": encT, "w2t": w2t, "vT": vT, "cT": cT, "negM": neg_m}
        )
    return in_maps


_NC_CACHE = {}


def _get_nc():
    if "nc" not in _NC_CACHE:
        _NC_CACHE["nc"] = build()
    return _NC_CACHE["nc"]


def kernel(hidden, encoder_outputs, attn_W, attn_b, v_W):
    nc = _get_nc()
    in_maps = shard_inputs(hidden, encoder_outputs, attn_W, attn_b, v_W)
    res = run_bass_kernel_spmd(nc, in_maps, list(range(NCORES)))
    context = np.concatenate(
        [r["ctx_out"].transpose(0, 2, 1).reshape(r["ctx_out"].shape[0], -1)
         / r["l_out"]
         for r in res.results],
        axis=0,
    )
    attn_weights = np.concatenate([r["aw_out"] for r in res.results], axis=0)
    return context, attn_weights


# revision 23
# speedup vs baseline: 267.4531x; 1.8390x over previous
"""Bahdanau attention on 8 Trainium2 NeuronCores (Bass/Tile, SPMD).

Single-pass design (per batch b, per seq chunk):
    energyT = W2 @ enc_chunk.T             [128h x 512s] x 8 m-blocks  (PE, f32r)
    tanhT   = tanh(energyT + c[b])         c = W1 @ hid + attn_b, host-computed;
                                           per-partition ACT bias, PSUM->SBUF
    scores  = v @ tanhT                    M=1 PE matmul, K-accumulated
    w       = exp(scores - M)              M = sum|v| >= max|scores| (fixed
                                           softmax shift -- no running max)
    wb      = ones @ w                     PE broadcast to 128 partitions
    acc    += enc_chunk.T * wb (sum s)     DVE tensor_tensor_reduce, chained
                                           via the initial-value operand
  end of batch: l = sum exp(srow - M); context = acc/l; attn_w = exp(srow-M)/l

One streaming pass over enc (encT layout only), no second enc read, no
collectives. Data-parallel over batch (32 -> 4 per core), weights replicated.
All matmul operands are float32r (TF32-like; full PE rate at N>=256).
PE-blocking ops that depend on slow ACT chains (wb, recip broadcast) are
deferred one energy-group to keep the PE queue stall-free.
"""

from collections import deque

import numpy as np

import concourse.bass as bass
import concourse.mybir as mybir
import concourse.tile as tile
from concourse import bacc
from concourse.bass_utils import run_bass_kernel_spmd

B, S, H = 32, 2048, 1024
NCORES = 8
BL = B // NCORES      # local batches per core
SC = 512              # seq chunk for pass 1 (matmul moving dim)
F32 = mybir.dt.float32
F32R = mybir.dt.float32r
AX = mybir.AxisListType
AF = mybir.ActivationFunctionType


def build(bl=BL, s=S, h=H, sc=SC):
    """Build the per-core Bass module. Dims: h%128==0, s%sc==0, sc<=512."""
    jm = h // 128          # number of 128-blocks of the hidden dim
    nsc = s // sc          # seq chunks

    nc = bacc.Bacc("TRN2", target_bir_lowering=False, debug=False)

    encT = nc.declare_dram_parameter("encT", [bl, nsc, jm, 128, sc], F32R, isOutput=False)
    w2t = nc.declare_dram_parameter("w2t", [jm, 128, h], F32R, isOutput=False)
    vT = nc.declare_dram_parameter("vT", [jm, 128], F32R, isOutput=False)
    cT = nc.declare_dram_parameter("cT", [jm, 128, bl], F32, isOutput=False)
    negM = nc.declare_dram_parameter("negM", [1, 1], F32, isOutput=False)
    ctx_out = nc.declare_dram_parameter("ctx_out", [bl, 128, jm], F32, isOutput=True)
    aw_out = nc.declare_dram_parameter("aw_out", [bl, s], F32, isOutput=True)
    l_out = nc.declare_dram_parameter("l_out", [bl, 1], F32, isOutput=True)

    with tile.TileContext(nc) as tc:
        with (
            tc.tile_pool(name="persist", bufs=1) as sb1,
            tc.tile_pool(name="et", bufs=5) as pool_et,
            tc.tile_pool(name="th", bufs=2) as pool_th,
            tc.tile_pool(name="rows", bufs=2) as pool_rw,
            tc.tile_pool(name="ps_e", bufs=2, space="PSUM") as pool_e,
            tc.tile_pool(name="ps_s", bufs=2, space="PSUM") as pool_s,
            tc.tile_pool(name="ps_w", bufs=2, space="PSUM") as pool_w,
        ):
            # ---- weights / constants resident in SBUF ----
            w2t_sb = sb1.tile([128, jm, h], F32R)
            nc.sync.dma_start(out=w2t_sb, in_=w2t.ap().rearrange("j p h -> p j h"))
            vT_sb = sb1.tile([128, jm], F32R)
            nc.scalar.dma_start(out=vT_sb, in_=vT.ap().rearrange("j p -> p j"))
            c_t = sb1.tile([128, jm, bl], F32)
            nc.scalar.dma_start(out=c_t, in_=cT.ap().rearrange("j p b -> p j b"))
            negm_sb = sb1.tile([1, 1], F32)
            nc.scalar.dma_start(out=negm_sb, in_=negM.ap())
            ones_f32 = sb1.tile([1, 128], F32)
            nc.vector.memset(ones_f32, 1.0)
            ones_sb = sb1.tile([1, 128], F32R)
            nc.vector.tensor_copy(ones_sb, ones_f32)

            # PE ops gated on slow ACT/DVE chains are emitted one
            # energy-group later so the in-order PE queue never stalls.
            deferred = deque()

            def drain_one():
                if deferred:
                    deferred.popleft()()

            def flush_deferred():
                while deferred:
                    deferred.popleft()()

            def energy_chunk(b, sci, srow):
                """Emit energy GEMM + tanh + scores for (b, chunk); returns
                (et, wexp) used by the deferred accumulation step."""
                et = pool_et.tile([128, jm, sc], F32R, name="et")
                nc.sync.dma_start(
                    out=et, in_=encT.ap()[b, sci].rearrange("j p s -> p j s")
                )
                th = pool_th.tile([128, jm, sc], F32R, name="th")
                s_ps = pool_s.tile([1, sc], F32, tag="scores", name="s_ps")
                for m in range(jm):
                    e_ps = pool_e.tile([128, sc], F32, tag="energy", name="e_ps")
                    for j in range(jm):
                        nc.tensor.matmul(
                            e_ps,
                            w2t_sb[:, j, m * 128 : (m + 1) * 128],
                            et[:, j, :],
                            start=(j == 0),
                            stop=(j == jm - 1),
                        )
                    if m > 0:
                        nc.tensor.matmul(
                            s_ps, vT_sb[:, m - 1 : m], th[:, m - 1, :],
                            start=(m - 1 == 0), stop=False,
                        )
                        drain_one()
                    nc.scalar.activation(
                        th[:, m, :], e_ps, AF.Tanh, bias=c_t[:, m, b : b + 1]
                    )
                nc.tensor.matmul(
                    s_ps, vT_sb[:, jm - 1 : jm], th[:, jm - 1, :],
                    start=(jm == 1), stop=True,
                )
                nc.vector.tensor_copy(srow[:, sci * sc : (sci + 1) * sc], s_ps)
                # unnormalized softmax weights for this chunk
                wexp = pool_rw.tile([1, sc], F32R, tag="wexp", name="wexp")
                nc.scalar.activation(wexp, s_ps, AF.Exp, bias=negm_sb)
                return et, wexp

            def accum_chunk(et, wexp, acc, scratch):
                """Deferred: broadcast w to 128 partitions (PE), then fold the
                chunk into the context accumulator (DVE mult/reduce/add)."""
                wb_ps = pool_w.tile([128, sc], F32, tag="wb", name="wb_ps")

                def emit():
                    nc.tensor.matmul(wb_ps, ones_sb, wexp, start=True, stop=True)
                    for j in range(jm):
                        nc.vector.tensor_mul(
                            scratch, et[:, j, :].bitcast(F32), wb_ps
                        )
                        part = pool_rw.tile([128, 1], F32, tag="part", name="part")
                        nc.vector.reduce_sum(part, scratch, axis=AX.X)
                        nc.vector.tensor_add(
                            acc[:, j : j + 1], acc[:, j : j + 1], part
                        )

                deferred.append(emit)

            def finish_batch(b, srow, acc):
                """Softmax epilogue. The context stays unnormalized on device;
                the host divides by l (exact fp32, one scalar per batch)."""
                aw_row = pool_rw.tile([1, s], F32, tag="aw", name="aw_row")
                l_sum = pool_rw.tile([1, 1], F32, tag="lsum", name="l_sum")
                nc.scalar.activation(aw_row, srow, AF.Exp, bias=negm_sb, accum_out=l_sum)
                rec = pool_rw.tile([1, 1], F32, tag="rec", name="rec")
                nc.vector.reciprocal(rec, l_sum)
                nc.vector.tensor_scalar_mul(aw_row, aw_row, rec)
                nc.sync.dma_start(out=aw_out.ap()[b : b + 1, :], in_=aw_row)
                nc.sync.dma_start(out=l_out.ap()[b : b + 1, :], in_=l_sum)

                def emit():
                    nc.sync.dma_start(out=ctx_out.ap()[b], in_=acc)

                deferred.append(emit)

            for b in range(bl):
                srow = pool_rw.tile([1, s], F32, tag="srow", name="srow")
                scratch = pool_rw.tile([128, sc], F32, tag="scr", name="scratch")
                acc = pool_rw.tile([128, jm], F32, tag="acc", name="acc")
                nc.vector.memset(acc, 0.0)
                for sci in range(nsc):
                    et, wexp = energy_chunk(b, sci, srow)
                    accum_chunk(et, wexp, acc, scratch)
                finish_batch(b, srow, acc)
            flush_deferred()

    nc.finalize()
    return nc


def shard_inputs(hidden, encoder_outputs, attn_W, attn_b, v_W, bl=BL, sc=SC):
    """Host-side prep: slice per core + lay out for the device program.
    Also computes the tiny decoder projection c = W1 @ hid + attn_b (0.05%
    of total FLOPs) and the softmax shift M = sum|v|."""
    hidden = np.ascontiguousarray(np.asarray(hidden, dtype=np.float32))
    enc = np.asarray(encoder_outputs, dtype=np.float32)
    attn_W = np.asarray(attn_W, dtype=np.float32)
    attn_b = np.ascontiguousarray(np.asarray(attn_b, dtype=np.float32))
    v_W = np.ascontiguousarray(np.asarray(v_W, dtype=np.float32))
    s, b_total, h = enc.shape
    jm = h // 128
    ncores = b_total // bl

    w2t = np.ascontiguousarray(attn_W.T[h:].reshape(jm, 128, h))
    vT = v_W[0].reshape(jm, 128)
    c_all = hidden @ attn_W[:, :h].T + attn_b          # [b_total, h]
    neg_m = -np.abs(v_W).sum(dtype=np.float32).reshape(1, 1)

    in_maps = []
    for c in range(ncores):
        bs = slice(c * bl, (c + 1) * bl)
        enc_c = enc[:, bs, :]                          # [s, bl, h]
        encT = np.ascontiguousarray(
            enc_c.reshape(s // sc, sc, bl, jm, 128).transpose(2, 0, 3, 4, 1)
        )                                              # [bl, nsc, jm, 128, sc]
        cT = np.ascontiguousarray(c_all[bs].T.reshape(jm, 128, bl))
        in_maps.append(
            {"encT": encT, "w2t": w2t, "vT": vT, "cT": cT, "negM": neg_m}
        )
    return in_maps


_NC_CACHE = {}


def _get_nc():
    if "nc" not in _NC_CACHE:
        _NC_CACHE["nc"] = build()
    return _NC_CACHE["nc"]


def kernel(hidden, encoder_outputs, attn_W, attn_b, v_W):
    nc = _get_nc()
    in_maps = shard_inputs(hidden, encoder_outputs, attn_W, attn_b, v_W)
    res = run_bass_kernel_spmd(nc, in_maps, list(range(NCORES)))
    context = np.concatenate(
        [r["ctx_out"].transpose(0, 2, 1).reshape(r["ctx_out"].shape[0], -1)
         / r["l_out"]
         for r in res.results],
        axis=0,
    )
    attn_weights = np.concatenate([r["aw_out"] for r in res.results], axis=0)
    return context, attn_weights
